# revision 7
# baseline (speedup 1.0000x reference)
"""Trainium2 Bass kernel for nn_M3Site (dual FunICross + gated fusion + BN head).

Sharding: data-parallel over the 16 graphs -> 2 graphs per core on 8 cores.
BatchNorm batch stats are all-reduced across cores on device.

Layout: every activation kept transposed ([feature, seq]) so the chain needs no
on-device transposes; softmax denominators and LayerNorm statistics (partition-
axis reductions) are computed with ones-matmuls on the TensorEngine, which also
yields them pre-broadcast. bf16 matmuls, fp32 PSUM/stats.

SBUF slot plan (per-partition bytes, bufs=1 tags reused across stages):
  w0: eT input (24K)          w1: KpT -> Vp -> r2 -> s2T (24K)
  w2: vw stream -> q (24K)    w3: kw stream -> expT -> r2b -> oT (28K)
  w4: ctxT -> sq (24K)        ow2s: streamed ow2 column blocks (2x3K)
"""

import numpy as np
import ml_dtypes

import concourse.bass as bass
import concourse.tile as tile
import concourse.mybir as mybir
from concourse import bacc
from concourse.bass_utils import run_bass_kernel_spmd

BF16 = mybir.dt.bfloat16
F32 = mybir.dt.float32
AX = mybir.AxisListType.X
OP = mybir.AluOpType
ACT = mybir.ActivationFunctionType

G, LMAX, N_TOT = 16, 1024, 8192
E_DIM, S_DIM, H, FF, COND, NC_CLS = 1536, 256, 256, 128, 768, 7
NCORES = 8
GPC = G // NCORES
QC = LMAX // 512
ET, ST = E_DIM // 128, S_DIM // 128
KT = LMAX // 128
OT = (E_DIM + S_DIM) // 128
H2 = H // 128

_CACHE = {}


def _mm(nc, psum, pairs):
    n = len(pairs)
    for i, (l, r) in enumerate(pairs):
        nc.tensor.matmul(psum, l, r, start=(i == 0), stop=(i == n - 1))


def build_kernel():
    nc = bacc.Bacc("TRN2", target_bir_lowering=False, num_devices=NCORES)

    def din(name, shape, dt=BF16):
        return nc.dram_tensor(name, shape, dt, kind="ExternalInput")

    eT_d = din("eT", [GPC, E_DIM, LMAX])
    sT_d = din("sT", [GPC, S_DIM, LMAX])
    kw1_d = din("kw1", [E_DIM, S_DIM]); vw1_d = din("vw1", [E_DIM, S_DIM])
    ow1_d = din("ow1", [S_DIM, S_DIM])
    w1a1_d = din("w1a1", [S_DIM, FF]); w21_d = din("w21", [FF, S_DIM])
    kw2_d = din("kw2", [S_DIM, E_DIM]); vw2_d = din("vw2", [S_DIM, E_DIM])
    ow2_d = din("ow2", [E_DIM, E_DIM])
    w1a2_d = din("w1a2", [E_DIM, FF]); w22_d = din("w22", [FF, E_DIM])
    fc1w_d = din("fc1w", [E_DIM + S_DIM, H]); fc2w_d = din("fc2w", [H, NC_CLS])
    wfcw_d = din("wfcw", [2 * (E_DIM + S_DIM), 1])
    vecs32 = {}
    for nm, t in [("kb1", ST), ("ob1", ST), ("g11", ST), ("b11", ST),
                  ("bb21", ST), ("g21", ST), ("b21", ST),
                  ("kb2", ET), ("ob2", ET), ("g12", ET), ("b12", ET),
                  ("bb22", ET), ("g22", ET), ("b22", ET),
                  ("fc1b", H2), ("bng", H2), ("bnb", H2),
                  ("hc1", GPC), ("hc2", GPC)]:
        vecs32[nm] = din(nm + "_t", [128, t], F32)
    fc2b_d = din("fc2b_t", [128, 1], F32)
    wfcb_d = din("wfcb_t", [128, 1], F32)
    vb1B_d = din("vb1B", [128, S_DIM], F32)
    vb2B_d = din("vb2B", [128, E_DIM])
    ident_d = din("ident", [128, 128], F32)
    logits_d = nc.dram_tensor("logits", [GPC, LMAX, NC_CLS], F32, kind="ExternalOutput")
    probs_d = nc.dram_tensor("probs", [GPC, LMAX, NC_CLS], F32, kind="ExternalOutput")

    with tile.TileContext(nc) as tc:
        pp = tc.alloc_tile_pool(name="params", bufs=1)
        wk = tc.alloc_tile_pool(name="wk", bufs=1)
        ow2p = tc.alloc_tile_pool(name="ow2p", bufs=2)
        scr = tc.alloc_tile_pool(name="scr", bufs=1)
        po = tc.alloc_tile_pool(name="po", bufs=2)
        tiny = tc.alloc_tile_pool(name="tiny", bufs=8)
        half = tc.alloc_tile_pool(name="half", bufs=2)
        ps = tc.alloc_tile_pool(name="ps", bufs=8, space="PSUM")
        dram = tc.alloc_tile_pool(name="dram", bufs=2, space="DRAM")

        def load_res(d, rows, cols, tag):
            t = pp.tile([128, rows // 128, cols], BF16, tag=tag)
            nc.sync.dma_start(t[:], d[:].rearrange("(t p) n -> p t n", p=128))
            return t

        ow1 = load_res(ow1_d, S_DIM, S_DIM, "ow1")
        w1a1 = load_res(w1a1_d, S_DIM, FF, "w1a1")
        w21 = load_res(w21_d, FF, S_DIM, "w21")
        w1a2 = load_res(w1a2_d, E_DIM, FF, "w1a2")
        w22 = load_res(w22_d, FF, E_DIM, "w22")
        fc2w = load_res(fc2w_d, H, NC_CLS, "fc2w")
        wfcw = load_res(wfcw_d, 2 * (E_DIM + S_DIM), 1, "wfcw")

        vec = {}
        for nm, d in vecs32.items():
            t = pp.tile([128, d.shape[1]], F32, tag=f"v_{nm}")
            nc.sync.dma_start(t[:], d[:])
            vec[nm] = t
        fc2b = pp.tile([128, 1], F32, tag="fc2b"); nc.sync.dma_start(fc2b[:], fc2b_d[:])
        wfcb = pp.tile([128, 1], F32, tag="wfcb"); nc.sync.dma_start(wfcb[:], wfcb_d[:])
        vb1B = pp.tile([128, S_DIM], F32, tag="vb1B"); nc.sync.dma_start(vb1B[:], vb1B_d[:])
        vb2B = pp.tile([128, E_DIM], BF16, tag="vb2B"); nc.sync.dma_start(vb2B[:], vb2B_d[:])
        ident = pp.tile([128, 128], F32, tag="ident"); nc.sync.dma_start(ident[:], ident_d[:])
        onesb = pp.tile([128, 128], BF16, tag="onesb"); nc.vector.memset(onesb[:], 1.0)
        eps_sb = pp.tile([128, 1], F32, tag="eps"); nc.vector.memset(eps_sb[:], 1e-5)
        bnacc = pp.tile([128, 2 * H2], F32, tag="bnacc")
        out1_list = []

        def tln(r2, T, g_t, b_t, out_tag):
            """Transposed LayerNorm over T partition-tiles of features."""
            inv = 1.0 / (T * 128)
            sq = wk.tile([128, T, LMAX], BF16, tag="w4")
            for mt in range(T):
                nc.vector.tensor_mul(sq[:, mt], r2[:, mt], r2[:, mt])
            meanB = scr.tile([128, LMAX], F32, tag="fA")
            varB = scr.tile([128, LMAX], F32, tag="fB")
            for qc in range(QC):
                p1 = ps.tile([128, 512], F32, tag="ps")
                _mm(nc, p1, [(onesb[:], r2[:, mt, bass.ts(qc, 512)]) for mt in range(T)])
                nc.scalar.mul(meanB[:, bass.ts(qc, 512)], p1[:], inv)
                p2 = ps.tile([128, 512], F32, tag="ps")
                _mm(nc, p2, [(onesb[:], sq[:, mt, bass.ts(qc, 512)]) for mt in range(T)])
                m2 = half.tile([128, 512], F32, tag="m2t")
                nc.vector.tensor_mul(m2[:], meanB[:, bass.ts(qc, 512)], meanB[:, bass.ts(qc, 512)])
                nc.vector.scalar_tensor_tensor(varB[:, bass.ts(qc, 512)], p2[:], inv, m2[:],
                                               OP.mult, OP.subtract)
            nc.scalar.activation(varB[:], varB[:], ACT.Sqrt, bias=eps_sb[:])
            nc.vector.reciprocal(varB[:], varB[:])  # now rstdB
            out = wk.tile([128, T, LMAX], BF16, tag=out_tag)
            tmp = scr.tile([128, LMAX], F32, tag="fC")
            for mt in range(T):
                nc.vector.tensor_sub(tmp[:], r2[:, mt], meanB[:])
                nc.vector.tensor_mul(tmp[:], tmp[:], varB[:])
                nc.vector.tensor_scalar(out[:, mt], tmp[:], g_t[:, mt:mt + 1], b_t[:, mt:mt + 1],
                                        OP.mult, OP.add)
            return out

        def funicross(qT, kvT, QT_, KVT_, kw_d, vw_d, ow_res, ow_dram, w1a, w2t, vbB, d1,
                      kb_t, ob_t, g1_t, b1_t, hc_col, bb2_t, g2_t, b2_t, out_tag):
            D1 = QT_ * 128
            NW = max(1, D1 // 512)
            VW = min(512, D1)
            # stream kw into w3 (dies before expT)
            kw = wk.tile([128, KVT_, D1], BF16, tag="w3")
            nc.sync.dma_start(kw[:], kw_d[:].rearrange("(t p) n -> p t n", p=128))
            KpT = wk.tile([128, QT_, LMAX], BF16, tag="w1")
            for mt in range(QT_):
                for qc in range(QC):
                    p = ps.tile([128, 512], F32, tag="ps")
                    _mm(nc, p, [(kw[:, kk, bass.ts(mt, 128)], kvT[:, kk, bass.ts(qc, 512)])
                                for kk in range(KVT_)])
                    nc.vector.tensor_scalar(KpT[:, mt, bass.ts(qc, 512)], p[:],
                                            kb_t[:, mt:mt + 1], None, OP.add)
            # scoresT -> expT
            expT = wk.tile([128, KT, LMAX], BF16, tag="w3")
            scale = 1.0 / float(np.sqrt(d1))
            for kt in range(KT):
                for qc in range(QC):
                    p = ps.tile([128, 512], F32, tag="ps")
                    _mm(nc, p, [(KpT[:, mt, bass.ts(kt, 128)], qT[:, mt, bass.ts(qc, 512)])
                                for mt in range(QT_)])
                    nc.scalar.activation(expT[:, kt, bass.ts(qc, 512)], p[:], ACT.Exp, scale=scale)
            invden = scr.tile([128, LMAX], F32, tag="fA")
            for qc in range(QC):
                p = ps.tile([128, 512], F32, tag="ps")
                _mm(nc, p, [(onesb[:], expT[:, kt, bass.ts(qc, 512)]) for kt in range(KT)])
                nc.vector.reciprocal(invden[:, bass.ts(qc, 512)], p[:])
            # Vp (keys on partitions); vw streamed into w2
            vw = wk.tile([128, KVT_, D1], BF16, tag="w2")
            nc.sync.dma_start(vw[:], vw_d[:].rearrange("(t p) n -> p t n", p=128))
            Vp = wk.tile([128, KT, D1], BF16, tag="w1")
            for kt in range(KT):
                for nck in range(NW):
                    p = ps.tile([128, 512], F32, tag="ps")
                    _mm(nc, p[:, :VW], [(kvT[:, kk, bass.ts(kt, 128)],
                                         vw[:, kk, bass.ds(nck * 512, VW)]) for kk in range(KVT_)])
                    nc.vector.tensor_add(Vp[:, kt, bass.ds(nck * 512, VW)], p[:, :VW],
                                         vbB[:, bass.ds(nck * 512, VW)])
            # ctxT' = Vp^T(keys) @ expT
            ctxT = wk.tile([128, QT_, LMAX], BF16, tag="w4")
            for nt in range(QT_):
                for qc in range(QC):
                    p = ps.tile([128, 512], F32, tag="ps")
                    _mm(nc, p, [(Vp[:, kt, bass.ts(nt, 128)], expT[:, kt, bass.ts(qc, 512)])
                                for kt in range(KT)])
                    nc.vector.tensor_copy(ctxT[:, nt, bass.ts(qc, 512)], p[:])
            # attnT = ow^T @ ctxT ; r2 = attnT*invden + ob + qT
            r2 = wk.tile([128, QT_, LMAX], BF16, tag="w1")
            for mt in range(QT_):
                if ow_res is not None:
                    ow_mt = None
                else:
                    ow_mt = ow2p.tile([128, QT_, 128], BF16, tag="ow2s")
                    nc.sync.dma_start(ow_mt[:], ow_dram[:].rearrange("(t p) n -> p t n", p=128)
                                      [:, :, bass.ts(mt, 128)])
                for qc in range(QC):
                    p = ps.tile([128, 512], F32, tag="ps")
                    if ow_res is not None:
                        _mm(nc, p, [(ow_res[:, nt, bass.ts(mt, 128)],
                                     ctxT[:, nt, bass.ts(qc, 512)]) for nt in range(QT_)])
                    else:
                        _mm(nc, p, [(ow_mt[:, nt, :], ctxT[:, nt, bass.ts(qc, 512)])
                                    for nt in range(QT_)])
                    t = half.tile([128, 512], F32, tag="attn_t")
                    nc.vector.tensor_mul(t[:], p[:], invden[:, bass.ts(qc, 512)])
                    nc.vector.scalar_tensor_tensor(r2[:, mt, bass.ts(qc, 512)], t[:],
                                                   ob_t[:, mt:mt + 1], qT[:, mt, bass.ts(qc, 512)],
                                                   OP.add, OP.add)
            q1 = tln(r2, QT_, g1_t, b1_t, "w2")
            # FFN
            hT = scr.tile([128, LMAX], BF16, tag="hT")
            for qc in range(QC):
                p = ps.tile([128, 512], F32, tag="ps")
                _mm(nc, p, [(w1a[:, mt, :], q1[:, mt, bass.ts(qc, 512)]) for mt in range(QT_)])
                nc.scalar.activation(hT[:, bass.ts(qc, 512)], p[:], ACT.Relu, bias=hc_col)
            r2b = wk.tile([128, QT_, LMAX], BF16, tag="w3")
            for mt in range(QT_):
                for qc in range(QC):
                    p = ps.tile([128, 512], F32, tag="ps")
                    nc.tensor.matmul(p, w2t[:, 0, bass.ts(mt, 128)], hT[:, bass.ts(qc, 512)],
                                     start=True, stop=True)
                    t = half.tile([128, 512], F32, tag="ff_t")
                    nc.vector.tensor_scalar(t[:], p[:], bb2_t[:, mt:mt + 1], None, OP.add)
                    nc.vector.tensor_add(r2b[:, mt, bass.ts(qc, 512)], t[:],
                                         q1[:, mt, bass.ts(qc, 512)])
            return tln(r2b, QT_, g2_t, b2_t, out_tag)

        for g in range(GPC):
            eT = wk.tile([128, ET, LMAX], BF16, tag="w0")
            nc.sync.dma_start(eT[:], eT_d[g].rearrange("(t p) q -> p t q", p=128))
            sT = wk.tile([128, ST, LMAX], BF16, tag="sTg")
            nc.sync.dma_start(sT[:], sT_d[g].rearrange("(t p) q -> p t q", p=128))

            s1T = funicross(sT, eT, ST, ET, kw1_d, vw1_d, ow1, None, w1a1, w21, vb1B, S_DIM,
                            vec["kb1"], vec["ob1"], vec["g11"], vec["b11"],
                            vec["hc1"][:, g:g + 1], vec["bb21"], vec["g21"], vec["b21"], "s1T")
            s2T = funicross(eT, sT, ET, ST, kw2_d, vw2_d, None, ow2_d, w1a2, w22, vb2B, E_DIM,
                            vec["kb2"], vec["ob2"], vec["g12"], vec["b12"],
                            vec["hc2"][:, g:g + 1], vec["bb22"], vec["g22"], vec["b22"], "w1")

            # gate
            wB = scr.tile([128, LMAX], F32, tag="fB")
            wrow = half.tile([1, LMAX], F32, tag="wrow")
            comb = ([s1T[:, t] for t in range(ST)] + [s2T[:, t] for t in range(ET)]
                    + [eT[:, t] for t in range(ET)] + [sT[:, t] for t in range(ST)])
            for qc in range(QC):
                p = ps.tile([128, 512], F32, tag="ps")
                _mm(nc, p[0:1, :], [(wfcw[:, ct, :], cb[:, bass.ts(qc, 512)])
                                    for ct, cb in enumerate(comb)])
                nc.scalar.activation(wrow[:, bass.ts(qc, 512)], p[0:1, :], ACT.Sigmoid,
                                     bias=wfcb[0:1])
                nc.gpsimd.partition_broadcast(wB[:, bass.ts(qc, 512)], wrow[:, bass.ts(qc, 512)])
            # gated features
            oTt = wk.tile([128, OT, LMAX], BF16, tag="w3")
            dtmp = scr.tile([128, LMAX], F32, tag="fC")
            for ct in range(OT):
                fT = s1T[:, ct] if ct < ST else s2T[:, ct - ST]
                tT = eT[:, ct] if ct < ET else sT[:, ct - ET]
                nc.vector.tensor_sub(dtmp[:], fT, tT)
                nc.vector.tensor_mul(dtmp[:], dtmp[:], wB[:])
                nc.vector.tensor_add(oTt[:, ct], dtmp[:], tT)
            # fc1 (weights streamed into w2, free after q2 died)
            fc1w = wk.tile([128, OT, H], BF16, tag="w2")
            nc.sync.dma_start(fc1w[:], fc1w_d[:].rearrange("(t p) n -> p t n", p=128))
            out1 = po.tile([128, H2, LMAX], BF16, tag="out1")
            out1_list.append(out1)
            for m2 in range(H2):
                for qc in range(QC):
                    p = ps.tile([128, 512], F32, tag="ps")
                    _mm(nc, p, [(fc1w[:, ct, bass.ts(m2, 128)], oTt[:, ct, bass.ts(qc, 512)])
                                for ct in range(OT)])
                    nc.vector.tensor_scalar(out1[:, m2, bass.ts(qc, 512)], p[:],
                                            vec["fc1b"][:, m2:m2 + 1], None, OP.add)
            # BN partials
            sqt = scr.tile([128, LMAX], F32, tag="fC")
            for m2 in range(H2):
                red = tiny.tile([128, 1], F32, tag="bnred")
                nc.vector.reduce_sum(red[:], out1[:, m2], axis=AX)
                if g == 0:
                    nc.vector.tensor_copy(bnacc[:, m2:m2 + 1], red[:])
                else:
                    nc.vector.tensor_add(bnacc[:, m2:m2 + 1], bnacc[:, m2:m2 + 1], red[:])
                nc.vector.tensor_mul(sqt[:], out1[:, m2], out1[:, m2])
                red2 = tiny.tile([128, 1], F32, tag="bnred2")
                nc.vector.reduce_sum(red2[:], sqt[:], axis=AX)
                cix = H2 + m2
                if g == 0:
                    nc.vector.tensor_copy(bnacc[:, cix:cix + 1], red2[:])
                else:
                    nc.vector.tensor_add(bnacc[:, cix:cix + 1], bnacc[:, cix:cix + 1], red2[:])

        ib = dram.tile([128, 2 * H2], F32)
        ob_ = dram.tile([128, 2 * H2], F32)
        nc.gpsimd.dma_start(ib[:], bnacc[:])
        nc.gpsimd.collective_compute("AllReduce", OP.add,
                                     replica_groups=[list(range(NCORES))],
                                     ins=[ib[:].opt()], outs=[ob_[:].opt()])
        ar = pp.tile([128, 2 * H2], F32, tag="ar")
        nc.gpsimd.dma_start(ar[:], ob_[:])
        inv_n = 1.0 / float(G * LMAX)
        scale_t, shift_t = [], []
        for m2 in range(H2):
            mu = tiny.tile([128, 1], F32, tag="mu")
            nc.vector.tensor_scalar(mu[:], ar[:, m2:m2 + 1], inv_n, None, OP.mult)
            var = tiny.tile([128, 1], F32, tag="var")
            nc.vector.tensor_scalar(var[:], ar[:, H2 + m2:H2 + m2 + 1], inv_n, None, OP.mult)
            mu2 = tiny.tile([128, 1], F32, tag="mu2")
            nc.vector.tensor_mul(mu2[:], mu[:], mu[:])
            nc.vector.tensor_sub(var[:], var[:], mu2[:])
            nc.scalar.activation(var[:], var[:], ACT.Sqrt, bias=eps_sb[:])
            nc.vector.reciprocal(var[:], var[:])
            sc = pp.tile([128, 1], F32, tag=f"bnsc{m2}")
            nc.vector.tensor_mul(sc[:], var[:], vec["bng"][:, m2:m2 + 1])
            sh = pp.tile([128, 1], F32, tag=f"bnsh{m2}")
            nc.vector.tensor_mul(sh[:], mu[:], sc[:])
            nc.vector.tensor_sub(sh[:], vec["bnb"][:, m2:m2 + 1], sh[:])
            scale_t.append(sc); shift_t.append(sh)

        for g in range(GPC):
            out1 = out1_list[g]
            a2 = out1
            for m2 in range(H2):
                nc.scalar.activation(a2[:, m2], out1[:, m2], ACT.Relu,
                                     bias=shift_t[m2][:], scale=scale_t[m2][:])
            lgT = scr.tile([128, LMAX], F32, tag="fA")
            for qc in range(QC):
                p = ps.tile([128, 512], F32, tag="ps")
                _mm(nc, p[0:NC_CLS, :], [(fc2w[:, m2, :], a2[:, m2, bass.ts(qc, 512)])
                                         for m2 in range(H2)])
                nc.vector.tensor_scalar(lgT[0:NC_CLS, bass.ts(qc, 512)], p[0:NC_CLS, :],
                                        fc2b[0:NC_CLS], None, OP.add)
            for qt in range(KT):
                tp = ps.tile([128, 512], F32, tag="ps")
                nc.tensor.transpose(tp[:, 0:NC_CLS], lgT[0:NC_CLS, bass.ts(qt, 128)],
                                    ident[0:NC_CLS, 0:NC_CLS])
                lg = tiny.tile([128, NC_CLS], F32, tag="lg")
                nc.vector.tensor_copy(lg[:], tp[:, 0:NC_CLS])
                nc.sync.dma_start(logits_d[g, bass.ts(qt, 128), :], lg[:])
                mx = tiny.tile([128, 1], F32, tag="mx")
                nc.vector.reduce_max(mx[:], lg[:], axis=AX)
                nmx = tiny.tile([128, 1], F32, tag="nmx")
                nc.scalar.mul(nmx[:], mx[:], -1.0)
                ex = tiny.tile([128, NC_CLS], F32, tag="ex")
                nc.scalar.activation(ex[:], lg[:], ACT.Exp, bias=nmx[:])
                sme = tiny.tile([128, 1], F32, tag="sme")
                nc.vector.reduce_sum(sme[:], ex[:], axis=AX)
                nc.vector.reciprocal(sme[:], sme[:])
                pr = tiny.tile([128, NC_CLS], F32, tag="pr")
                nc.vector.tensor_scalar(pr[:], ex[:], sme[:], None, OP.mult)
                nc.sync.dma_start(probs_d[g, bass.ts(qt, 128), :], pr[:])

        for pool in (dram, ps, half, tiny, po, scr, ow2p, wk, pp):
            pool.release()

    nc.compile()
    return nc


def _tvec(v, t):
    return np.ascontiguousarray(np.asarray(v, np.float32).reshape(t, 128).T)


def kernel(esm_rep, egnn_output, func, batch, ptr, y, p1, p2, ph):
    esm_rep = np.asarray(esm_rep, np.float32)
    egnn_output = np.asarray(egnn_output, np.float32)
    func = np.asarray(func, np.float32)
    batch = np.asarray(batch).astype(np.int64)
    ptr = np.asarray(ptr).astype(np.int64)
    y = np.asarray(y).astype(np.int64)
    p1 = {k: np.asarray(v, np.float32) for k, v in p1.items()}
    p2 = {k: np.asarray(v, np.float32) for k, v in p2.items()}
    ph = {k: np.asarray(v, np.float32) for k, v in ph.items()}
    bf = ml_dtypes.bfloat16

    if "nc" not in _CACHE:
        _CACHE["nc"] = build_kernel()
    nc = _CACHE["nc"]

    eT_all = np.zeros((G, E_DIM, LMAX), bf)
    sT_all = np.zeros((G, S_DIM, LMAX), bf)
    for g in range(G):
        a, b = int(ptr[g]), int(ptr[g + 1])
        eT_all[g, :, :b - a] = esm_rep[a:b].T.astype(bf)
        sT_all[g, :, :b - a] = egnn_output[a:b].T.astype(bf)

    hc1_all = (func @ p1["w1"][S_DIM:] + p1["bb1"]).astype(np.float32)
    hc2_all = (func @ p2["w1"][E_DIM:] + p2["bb1"]).astype(np.float32)

    com = {
        "kw1": p1["kw"].astype(bf), "vw1": p1["vw"].astype(bf), "ow1": p1["ow"].astype(bf),
        "w1a1": p1["w1"][:S_DIM].astype(bf), "w21": p1["w2"].astype(bf),
        "kw2": p2["kw"].astype(bf), "vw2": p2["vw"].astype(bf), "ow2": p2["ow"].astype(bf),
        "w1a2": p2["w1"][:E_DIM].astype(bf), "w22": p2["w2"].astype(bf),
        "fc1w": ph["fc1w"].astype(bf), "fc2w": ph["fc2w"].astype(bf),
        "wfcw": ph["wfcw"].astype(bf),
        "kb1_t": _tvec(p1["kb"], ST), "ob1_t": _tvec(p1["ob"], ST),
        "g11_t": _tvec(p1["g1"], ST), "b11_t": _tvec(p1["b1"], ST),
        "bb21_t": _tvec(p1["bb2"], ST), "g21_t": _tvec(p1["g2"], ST), "b21_t": _tvec(p1["b2"], ST),
        "kb2_t": _tvec(p2["kb"], ET), "ob2_t": _tvec(p2["ob"], ET),
        "g12_t": _tvec(p2["g1"], ET), "b12_t": _tvec(p2["b1"], ET),
        "bb22_t": _tvec(p2["bb2"], ET), "g22_t": _tvec(p2["g2"], ET), "b22_t": _tvec(p2["b2"], ET),
        "fc1b_t": _tvec(ph["fc1b"], H2), "bng_t": _tvec(ph["bng"], H2), "bnb_t": _tvec(ph["bnb"], H2),
        "fc2b_t": np.pad(ph["fc2b"], (0, 128 - NC_CLS)).reshape(128, 1).astype(np.float32),
        "wfcb_t": np.full((128, 1), float(np.asarray(ph["wfcb"]).reshape(-1)[0]), np.float32),
        "vb1B": np.tile(p1["vb"].astype(np.float32), (128, 1)),
        "vb2B": np.tile(p2["vb"], (128, 1)).astype(bf),
        "ident": np.eye(128, dtype=np.float32),
    }
    in_maps = []
    for c in range(NCORES):
        gsl = slice(c * GPC, (c + 1) * GPC)
        m = dict(com)
        m["eT"] = np.ascontiguousarray(eT_all[gsl])
        m["sT"] = np.ascontiguousarray(sT_all[gsl])
        m["hc1_t"] = np.ascontiguousarray(hc1_all[gsl].T)
        m["hc2_t"] = np.ascontiguousarray(hc2_all[gsl].T)
        in_maps.append(m)
    _CACHE["in_maps"] = in_maps

    res = run_bass_kernel_spmd(nc, in_maps, core_ids=list(range(NCORES)))
    logits_dense = np.concatenate([r["logits"] for r in res.results], axis=0)
    probs_dense = np.concatenate([r["probs"] for r in res.results], axis=0)

    pos = np.arange(N_TOT) - ptr[batch]
    recon = logits_dense[batch, pos]
    token_logits = probs_dense[batch, pos].astype(np.float32)

    c = ph["centers"]
    center_loss = np.float32(0.5 * np.mean(np.sum((recon - c[y]) ** 2, axis=-1)))
    diff = c[:, None, :] - c[None, :, :]
    dist = np.sqrt(np.sum(diff ** 2, axis=-1) + 1e-12)
    mask = np.triu(np.ones((NC_CLS, NC_CLS), np.float32), 1)
    inter_loss = np.float32(np.sum(np.maximum(0.1 - dist, 0.0) * mask) / np.sum(mask))
    return token_logits, center_loss, inter_loss
